# revision 22
# baseline (speedup 1.0000x reference)
"""Bass/Tile kernel builder for sharded LlamaAttention on TRN2 (bf16 v3).

Per-core problem (8 cores = 2 batch groups x 4 head groups):
  inputs (per core):
    xT    [D=2048, S=2048]  bf16  hidden_states[b].T
    wqT   [D=2048, 512]     bf16  wq rows for this core's 4 q heads, transposed
    wkT   [D=2048, 128]     bf16
    wvT   [D=2048, 128]     bf16
    woT   [512, D=2048]     bf16  wo cols for this core's heads, transposed
    cosT  [128, S=2048]     f32   cos[b].T
    sinT  [128, S=2048]     f32
  output:
    out   [S=2048, D=2048]  bf16  partial output (this head group's contribution)

All matmul operands bf16 (1 cycle/row); PSUM accumulation fp32.

Structure (PE kept dense to avoid HAM re-throttling):
 - Stage A h-major: each projection target finishes its contraction loop
   before the next starts, so its RoPE (DVE) overlaps the following
   projections; by stage A's end the DVE queue is drained -> no A->B stall.
 - V is projected directly in natural [s, hd] layout via N=128 matmuls with
   x-blocks stationary (no PE transposes, no identity, 2 fewer PSUM banks).
 - Stage B logits sT[k, q]; diagonal blocks narrowed to live columns.
   Denominator accumulates in bf16 on DVE/GpSimd (alternating); per-head
   epilogue (ones-matmul row-sum -> reciprocal_approx_fast -> broadcast
   matmul -> normalize) is software-pipelined one head behind so the PE
   never waits on the accumulation chain.
 - Stage C (output projection) interleaved per q-chunk.
"""

import sys
sys.path.insert(0, '/opt/trn_rl_repo')

from contextlib import ExitStack

import concourse.bass as bass
import concourse.tile as tile
import concourse.mybir as mybir
from concourse import bacc
from concourse.alu_op_type import AluOpType

F32 = mybir.dt.float32
BF16 = mybir.dt.bfloat16

S = 2048
D = 2048
HD = 128
NQH = 4            # q heads per core
SC = 512           # projection S-chunk width / attention q-chunk width
NSC = S // SC      # 4
ND = D // 128      # 16 contraction chunks
NQT = S // 128     # 16 q tiles
SCALE = HD ** -0.5
MASK_NEG = -1e12


def build_kernel():
    nc = bacc.Bacc(trn_type="TRN2", target_bir_lowering=False, debug=False,
                   num_devices=1)
    xT = nc.dram_tensor("xT", [D, S], BF16, kind="ExternalInput").ap()
    wqT = nc.dram_tensor("wqT", [D, NQH * HD], BF16, kind="ExternalInput").ap()
    wkT = nc.dram_tensor("wkT", [D, HD], BF16, kind="ExternalInput").ap()
    wvT = nc.dram_tensor("wvT", [D, HD], BF16, kind="ExternalInput").ap()
    woT = nc.dram_tensor("woT", [NQH * HD, D], BF16, kind="ExternalInput").ap()
    cosT = nc.dram_tensor("cosT", [HD, S], F32, kind="ExternalInput").ap()
    sinT = nc.dram_tensor("sinT", [HD, S], F32, kind="ExternalInput").ap()
    out = nc.dram_tensor("out", [S, D], BF16, kind="ExternalOutput").ap()

    with tile.TileContext(nc) as tc, ExitStack() as top:
        # ---------- resident pools ----------
        consts = top.enter_context(tc.tile_pool(name="consts", bufs=1))
        qkpool = top.enter_context(tc.tile_pool(name="qk", bufs=1))
        vpool = top.enter_context(tc.tile_pool(name="vnat", bufs=1))
        wopool = top.enter_context(tc.tile_pool(name="wo", bufs=1))
        avnpool = top.enter_context(tc.tile_pool(name="avn", bufs=1))

        ones_col = consts.tile([128, 1], BF16, tag="ones_col")
        nc.gpsimd.memset(ones_col[:], 1.0)
        ones_row = consts.tile([1, 128], BF16, tag="ones_row")
        nc.gpsimd.memset(ones_row[:], 1.0)
        # mext[r, cc] = 0 if cc >= 384 + r else MASK_NEG ; [128, 896]
        mext = consts.tile([128, 896], F32, tag="mext")
        nc.gpsimd.memset(mext[:], 0.0)
        nc.gpsimd.affine_select(
            out=mext[:], in_=mext[:],
            compare_op=AluOpType.is_ge,
            fill=MASK_NEG,
            base=-384,
            pattern=[[1, 896]],
            channel_multiplier=-1,
        )

        # q/k transposed+roped [HD, S] bf16; v natural per 512-chunk:
        # v_sb[sc][:, 128b:128b+128] = V rows [512sc+128b, +128) x hd
        qT_sb = [qkpool.tile([128, S], BF16, tag=f"qT{h}", name=f"qT{h}") for h in range(NQH)]
        kT_sb = qkpool.tile([128, S], BF16, tag="kT", name="kT")
        v_sb = [vpool.tile([128, SC], BF16, tag=f"v{i}", name=f"v{i}") for i in range(NSC)]
        woT_sb = [wopool.tile([128, D], BF16, tag=f"wo{h}", name=f"wo{h}") for h in range(NQH)]
        avn_sb = [avnpool.tile([128, S], BF16, tag=f"avn{h}", name=f"avn{h}")
                  for h in range(NQH)]

        # ================= Stage A: projections + rope =================
        ctxa = top.enter_context(ExitStack())
        wq_pool = ctxa.enter_context(tc.tile_pool(name="wq", bufs=ND))
        wkv_pool = ctxa.enter_context(tc.tile_pool(name="wkv", bufs=2))
        cs_pool = ctxa.enter_context(tc.tile_pool(name="cs", bufs=2))
        xt_pool = ctxa.enter_context(tc.tile_pool(name="xt", bufs=2 * ND))
        rs_pool = ctxa.enter_context(tc.tile_pool(name="ropes", bufs=2))
        pj_ps = ctxa.enter_context(
            tc.tile_pool(name="pj_ps", bufs=6, space="PSUM"))

        # startup DMAs split across engine queues so they land in parallel:
        # x on sync, wq on scalar, cos/sin on vector, wk/wv on gpsimd
        x_first = []
        for d in range(ND):
            x_t = xt_pool.tile([128, SC], BF16, tag="xt")
            nc.sync.dma_start(x_t[:], xT[bass.ts(d, 128), bass.ts(0, SC)])
            x_first.append(x_t)
        wq_t = [wq_pool.tile([128, NQH * HD], BF16, tag="wq", name="wqt") for _ in range(ND)]
        for d in range(ND):
            nc.scalar.dma_start(wq_t[d][:], wqT[bass.ts(d, 128), :])
        cos_t = cs_pool.tile([128, S], F32, tag="cs")
        nc.gpsimd.dma_start(cos_t[:], cosT[:, :])
        sin_t = cs_pool.tile([128, S], F32, tag="cs")
        nc.gpsimd.dma_start(sin_t[:], sinT[:, :])
        # wk/wv: [D,HD] rearranged into [128, ND*HD] (chunk d at cols d*HD)
        wk_t = wkv_pool.tile([128, ND * HD], BF16, tag="wkv")
        nc.gpsimd.dma_start(
            wk_t[:].rearrange("p (d h) -> p d h", h=HD),
            wkT.rearrange("(d p) h -> p d h", p=128))
        wv_t = wkv_pool.tile([128, ND * HD], BF16, tag="wkv")
        nc.gpsimd.dma_start(
            wv_t[:].rearrange("p (d h) -> p d h", h=HD),
            wvT.rearrange("(d p) h -> p d h", p=128))
        for h in range(NQH):
            nc.scalar.dma_start(woT_sb[h][:], woT[bass.ts(h, 128), :])

        def rope(dst_slice, x_ps, c_sl, s_sl, t1, t2):
            # dst = x*c + rot_half(x)*s   (x in PSUM f32, dst bf16)
            nc.vector.tensor_tensor(t1[:], x_ps[:], c_sl, AluOpType.mult)
            # t2[0:64] = -x[64:128]*s[0:64]
            nc.vector.scalar_tensor_tensor(
                t2[0:64, :], x_ps[64:128, :], -1.0, s_sl[0:64, :],
                op0=AluOpType.mult, op1=AluOpType.mult)
            # t2[64:128] = x[0:64]*s[64:128]
            nc.vector.tensor_tensor(
                t2[64:128, :], x_ps[0:64, :], s_sl[64:128, :], AluOpType.mult)
            nc.vector.tensor_tensor(dst_slice, t1[:], t2[:], AluOpType.add)

        for sc in range(NSC):
            ssl = bass.ts(sc, SC)
            if sc == 0:
                x_ts = x_first
            else:
                x_ts = []
                for d in range(ND):
                    x_t = xt_pool.tile([128, SC], BF16, tag="xt")
                    nc.sync.dma_start(x_t[:], xT[bass.ts(d, 128), ssl])
                    x_ts.append(x_t)
            # q heads, h-major so rope overlaps the next projection
            for h in range(NQH):
                q_ps = pj_ps.tile([128, SC], F32, tag="pj", name="qps")
                for d in range(ND):
                    nc.tensor.matmul(
                        q_ps[:], wq_t[d][:, bass.ts(h, HD)], x_ts[d][:],
                        start=(d == 0), stop=(d == ND - 1))
                t1 = rs_pool.tile([128, SC], F32, tag="t1")
                t2 = rs_pool.tile([128, SC], F32, tag="t2")
                rope(qT_sb[h][:, ssl], q_ps, cos_t[:, ssl], sin_t[:, ssl],
                     t1, t2)
            # v natural: x-blocks stationary, wv moving (N=128)
            v_ps = pj_ps.tile([128, SC], F32, tag="pj", name="vps")
            for b in range(SC // 128):
                for d in range(ND):
                    nc.tensor.matmul(
                        v_ps[:, bass.ts(b, 128)],
                        x_ts[d][:, bass.ts(b, 128)],
                        wv_t[:, bass.ts(d, HD)],
                        start=(d == 0), stop=(d == ND - 1))
            nc.scalar.copy(v_sb[sc][:], v_ps[:])
            # k last: its rope is the only DVE tail at stage A's end
            k_ps = pj_ps.tile([128, SC], F32, tag="pj", name="kps")
            for d in range(ND):
                nc.tensor.matmul(
                    k_ps[:], wk_t[:, bass.ts(d, HD)], x_ts[d][:],
                    start=(d == 0), stop=(d == ND - 1))
            t1 = rs_pool.tile([128, SC], F32, tag="t1")
            t2 = rs_pool.tile([128, SC], F32, tag="t2")
            rope(kT_sb[:, ssl], k_ps, cos_t[:, ssl], sin_t[:, ssl], t1, t2)

        # ================= Stages B + C (interleaved per q-chunk) ==========
        ctxa.close()
        sp_ps = top.enter_context(
            tc.tile_pool(name="sp_ps", bufs=2, space="PSUM"))
        av_ps = top.enter_context(
            tc.tile_pool(name="av_ps", bufs=2, space="PSUM"))
        dn_ps = top.enter_context(
            tc.tile_pool(name="dn_ps", bufs=1, space="PSUM"))
        o_ps = top.enter_context(
            tc.tile_pool(name="o_ps", bufs=3, space="PSUM"))
        p_pool = top.enter_context(tc.tile_pool(name="p_sb", bufs=4))
        d_pool = top.enter_context(tc.tile_pool(name="dacc", bufs=2))
        r_pool = top.enter_context(tc.tile_pool(name="recip", bufs=2))
        o_pool = top.enter_context(tc.tile_pool(name="o_sb", bufs=3))

        def emit_epilogue(h, qsl, av, dacc):
            # row-sum over k (ones matmul), 1/x, broadcast, normalize
            dnp = dn_ps.tile([128, SC], F32, tag="dn")
            nc.tensor.matmul(dnp[0:1, :], ones_col[:], dacc[:],
                             start=True, stop=True)
            rcp_h = r_pool.tile([1, SC], F32, tag="rcp")
            nc.vector.reciprocal_approx_fast(rcp_h[:], dnp[0:1, :])
            rbrow = r_pool.tile([1, SC], BF16, tag="rb")
            nc.vector.tensor_copy(rbrow[:], rcp_h[:])
            bc = o_ps.tile([128, SC], F32, tag="o")
            nc.tensor.matmul(bc[:], ones_row[:], rbrow[:],
                             start=True, stop=True)
            bc_sb = r_pool.tile([128, SC], F32, tag="bcs")
            nc.scalar.copy(bc_sb[:], bc[:])
            nc.vector.tensor_tensor(
                avn_sb[h][:, qsl], av[:], bc_sb[:], AluOpType.mult)

        def emit_stage_c(j):
            for t in range(4 * j, 4 * (j + 1)):
                o_sb = o_pool.tile([128, D], BF16, tag="o")
                for dc in range(D // SC):
                    op = o_ps.tile([128, SC], F32, tag="o")
                    for h in range(NQH):
                        nc.tensor.matmul(
                            op[:], avn_sb[h][:, bass.ts(t, 128)],
                            woT_sb[h][:, bass.ts(dc, SC)],
                            start=(h == 0), stop=(h == NQH - 1))
                    nc.vector.tensor_copy(o_sb[:, bass.ts(dc, SC)], op[:])
                nc.sync.dma_start(out[bass.ts(t, 128), :], o_sb[:])

        pending = None   # (h, qsl, av, dacc) of the previous head
        for j in range(NSC):          # q chunk [512j, 512j+512)
            qsl = bass.ts(j, SC)
            nkc = 4 * (j + 1)
            for h in range(NQH):
                av = av_ps.tile([128, SC], F32, tag="av")
                dacc = d_pool.tile([128, SC], BF16, tag="dacc")
                for kc in range(nkc):
                    m = kc - 4 * j
                    lo = 128 * m if m > 0 else 0   # live q cols [lo, 512)
                    st = sp_ps.tile([128, SC], F32, tag="st")
                    nc.tensor.matmul(
                        st[:, lo:SC], kT_sb[:, bass.ts(kc, 128)],
                        qT_sb[h][:, j * SC + lo: (j + 1) * SC],
                        start=True, stop=True)
                    if m >= 0:   # diagonal block: triangular mask
                        nc.vector.tensor_tensor(
                            st[:, lo:SC], st[:, lo:SC],
                            mext[:, 384: 896 - lo], AluOpType.add)
                    p = p_pool.tile([128, SC], BF16, tag="p")
                    nc.scalar.activation(
                        p[:, lo:SC], st[:, lo:SC],
                        mybir.ActivationFunctionType.Exp, scale=SCALE)
                    # denominator accumulation, bf16 (alternate DVE/GpSimd)
                    if kc == 0:
                        nc.vector.tensor_copy(dacc[:], p[:])
                    else:
                        eng = nc.vector if kc % 2 == 0 else nc.gpsimd
                        eng.tensor_tensor(dacc[:, lo:SC], dacc[:, lo:SC],
                                          p[:, lo:SC], AluOpType.add)
                    nc.tensor.matmul(
                        av[:, lo:SC], v_sb[kc // 4][:, bass.ts(kc % 4, 128)],
                        p[:, lo:SC],
                        start=(kc == 0), stop=(kc == nkc - 1))
                # previous head's epilogue lands here: its accumulation chain
                # finished during this head's kc loop, so the PE never stalls
                if pending is not None:
                    emit_epilogue(*pending)
                    # stage C of the previous chunk, one extra head late so
                    # its avn normalization (DVE chain) has fully drained
                    if h == 1 and j > 0:
                        emit_stage_c(j - 1)
                pending = (h, qsl, av, dacc)
        emit_epilogue(*pending)
        emit_stage_c(NSC - 1)

    nc.compile()
    return nc


# ======================================================================
# Entry point: full-input kernel with internal 8-core sharding
# ======================================================================

import numpy as np
import ml_dtypes

def _install_axon_hooks():
    """Recreate antenv.axon_hooks (absent in this env) so bass_utils works."""
    import types
    if 'antenv.axon_hooks' in sys.modules:
        return
    try:
        import antenv
    except ImportError:
        return
    mod = types.ModuleType('antenv.axon_hooks')
    _state = {'hook': None}
    mod.set_axon_ntff_profile_hook = lambda h: _state.__setitem__('hook', h)
    mod.get_axon_ntff_profile_hook = lambda: _state['hook']
    sys.modules['antenv.axon_hooks'] = mod
    antenv.axon_hooks = mod


_NC_CACHE = {}
_RUN_KWARGS = {}      # test harness may set {'trace': True}
_LAST_RESULTS = None  # test harness reads exec_time_ns / trace path from here


def _get_nc():
    if 'nc' not in _NC_CACHE:
        _NC_CACHE['nc'] = build_kernel()
    return _NC_CACHE['nc']


def kernel(**inputs):
    """LlamaAttention forward on 8 NeuronCores.

    Sharding: core c = (batch b = c // 4, head-group g = c % 4); each core
    computes 4 q-heads (1 kv head) for one batch element and its partial
    output through the corresponding wo columns; partials are summed on host.
    """
    _install_axon_hooks()
    from concourse import bass_utils

    BF = ml_dtypes.bfloat16
    hs = np.asarray(inputs["hidden_states"], np.float32)
    cos = np.asarray(inputs["cos"], np.float32)
    sin = np.asarray(inputs["sin"], np.float32)
    wq = np.asarray(inputs["wq"], np.float32).astype(BF)
    wk = np.asarray(inputs["wk"], np.float32).astype(BF)
    wv = np.asarray(inputs["wv"], np.float32).astype(BF)
    wo = np.asarray(inputs["wo"], np.float32).astype(BF)

    in_maps = []
    for c in range(8):
        b, g = c // 4, c % 4
        in_maps.append({
            "xT": np.ascontiguousarray(hs[b].T).astype(BF),
            "wqT": np.ascontiguousarray(wq[512 * g:512 * (g + 1), :].T),
            "wkT": np.ascontiguousarray(wk[128 * g:128 * (g + 1), :].T),
            "wvT": np.ascontiguousarray(wv[128 * g:128 * (g + 1), :].T),
            "woT": np.ascontiguousarray(wo[:, 512 * g:512 * (g + 1)].T),
            "cosT": np.ascontiguousarray(cos[b].T),
            "sinT": np.ascontiguousarray(sin[b].T),
        })

    nc = _get_nc()
    run_kwargs = dict(trace=False)
    run_kwargs.update(_RUN_KWARGS)
    res = bass_utils.run_bass_kernel_spmd(nc, in_maps, core_ids=list(range(8)),
                                          **run_kwargs)
    global _LAST_RESULTS
    _LAST_RESULTS = res
    outs = [np.asarray(res.results[c]["out"], np.float32) for c in range(8)]
    full = np.stack([outs[0] + outs[1] + outs[2] + outs[3],
                     outs[4] + outs[5] + outs[6] + outs[7]])
    return full.astype(np.float32)
